# revision 1
# baseline (speedup 1.0000x reference)
"""Bass/Tile TRN2 kernel for nn_Network_21131239096982 (gnn_message_passing).

Sharding: 8 cores = 4 samples x 2 x-halves. Each core computes the conv
stack for its (sample, x-half) shard; per-layer pair AllGather rebuilds the
full per-sample feature vector; a final 8-way AllGather assembles all
samples for the (batch-coupled) batchnorm MLP head, computed redundantly.

Key algebraic restructure vs the reference: the per-pair kernel tensor
R[b,x,y,i,j] is never materialized. The einsum
  f'[x,i] = sum_{y,j} R[x,y,i,j] f[y,j] m[y]
is reordered as  G[y,h,i] = sum_j w3r[h,i,j] fm[y,j]  (tiny) followed by
  f'[x,i] = sum_{y,h} s2'[x,y,h] G[y,h,i]  (+ constant row),
where s2' = ln(sigmoid(-5*pre2)) = -softplus(5*pre2) is the (negated)
radial-MLP hidden activation; all affine constants and signs are folded
into host-precomputed weights. softplus is computed as sigmoid -> ln on
the scalar engine (no native softplus table on TRN2).
"""

import math
import os

import numpy as np

B, N, EMB, MUL = 4, 128, 32, 32
NB, MAXR = 10, 10.0
HID, BETA = 128, 5.0
MID, OUT = 256, 128
NL = 4
Y0 = 1.0 / (2.0 * math.sqrt(math.pi))
XH = N // 2  # 64: x-half per core
NP = N * XH  # 8192 pairs per core, order (x outer, y inner)
NCORES = 8
STEP = MAXR / (NB - 1)
LN2 = math.log(2.0)

_cached = None


def _patch_ldw_opt():
    from concourse import bass_utils
    if getattr(bass_utils, "_ldwopt_patched", False):
        return
    orig = bass_utils.run_command

    def patched(argv, **kw):
        if os.environ.get("KERNEL_LDWOPT", "0") == "1":
            argv = ["--enable-ldw-opt=true" if a == "--enable-ldw-opt=false" else a
                    for a in argv]
        return orig(argv, **kw)

    bass_utils.run_command = patched
    bass_utils._ldwopt_patched = True


def _build():
    import jax

    jax.devices()  # axon boot
    from concourse import bacc, tile, mybir
    from concourse.tile import add_dep_helper
    _patch_ldw_opt()

    F32 = mybir.dt.float32
    F32R = mybir.dt.float32r
    BF16 = mybir.dt.bfloat16
    AF = mybir.ActivationFunctionType
    ALU = mybir.AluOpType

    # dtype knobs: G-stage + final contraction run in bf16
    G_DT = BF16
    g_mmdt = BF16

    nc = bacc.Bacc("TRN2", debug=False, num_devices=NCORES)

    def din(name, shape, dt=F32):
        return nc.dram_tensor(name, shape, dt, kind="ExternalInput").ap()

    geoYL_d = din("geoYL", [5, N])
    geoXR_d = din("geoXR", [5, XH])
    f0_d = din("f0", [N, EMB])
    maskcol_d = din("maskcol", [N, 1])
    maskxr_d = din("maskxr", [1, XH])
    gridb_d = din("gridb", [128, NB])
    ident_d = din("ident", [128, 128])
    ones1_d = din("ones1", [1, 128])
    ones128_d = din("ones128", [128, 1])
    w1f_d = din("w1f", [NB, NL * HID], F32R)
    b1f_d = din("b1f", [HID, NL])
    w2f_d = din("w2f", [HID, NL * HID], F32R)
    b2f_d = din("b2f", [HID, NL])
    wg_d = din("wg", [MUL, NL * MUL * HID], g_mmdt)
    w3c_d = din("w3c", [MUL, NL * MUL])
    w1c_d = din("w1c", [EMB, MID])
    b1c_d = din("b1c", [128, 2])
    w2c_d = din("w2c", [128, MID])
    b2c_d = din("b2c", [128, 1])
    g1r_d = din("g1r", [1, N])
    be1r_d = din("be1r", [1, N])
    g2r_d = din("g2r", [1, N])
    be2r_d = din("be2r", [1, N])
    maskrow_d = din("maskrow", [1, B * N])
    cvec_d = din("cvec", [128, 3])
    out_d = nc.dram_tensor("out", [B, OUT], F32, kind="ExternalOutput").ap()

    SQN = 1.0 / math.sqrt(N)

    with tile.TileContext(nc) as tc:
        with (
            tc.tile_pool(name="const", bufs=1) as cp,
            tc.tile_pool(name="slot", bufs=2) as slotp,
            tc.tile_pool(name="s2w", bufs=2) as s2wp,
            tc.tile_pool(name="big", bufs=1) as bigp,
            tc.tile_pool(name="work", bufs=2) as wp,
            tc.tile_pool(name="gw", bufs=1) as gwp,
            tc.tile_pool(name="ps_mm", bufs=4, space="PSUM") as ps_mm,
            tc.tile_pool(name="ps_small", bufs=2, space="PSUM") as ps_sm,
            tc.tile_pool(name="ps_g", bufs=2, space="PSUM") as ps_g,
            tc.tile_pool(name="dram", bufs=1, space="DRAM") as dp,
        ):
            # ---- constants to SBUF ----
            def cload(ap, shape, dt=F32, tag=""):
                t = cp.tile(shape, dt, name=tag or ap.tensor.name + "_sb")
                nc.sync.dma_start(t[:], ap[:])
                return t

            geoYL = cload(geoYL_d, [5, N])
            geoXR = cload(geoXR_d, [5, XH])
            f0sb = cload(f0_d, [N, EMB])
            maskcol = cload(maskcol_d, [N, 1])
            maskxr = cload(maskxr_d, [1, XH])
            gridb = cload(gridb_d, [128, NB])
            ident = cload(ident_d, [128, 128])
            ones1 = cload(ones1_d, [1, 128])
            ones128 = cload(ones128_d, [128, 1])
            w1f = cload(w1f_d, [NB, NL * HID], F32R)
            b1f = cload(b1f_d, [HID, NL])
            w2f = cload(w2f_d, [HID, NL * HID], F32R)
            b2f = cload(b2f_d, [HID, NL])
            w3c = cload(w3c_d, [MUL, NL * MUL])
            w1c = cload(w1c_d, [EMB, MID])
            b1c = cload(b1c_d, [128, 2])
            w2c = cload(w2c_d, [128, MID])
            b2c = cload(b2c_d, [128, 1])
            g1r = cload(g1r_d, [1, N])
            be1r = cload(be1r_d, [1, N])
            g2r = cload(g2r_d, [1, N])
            be2r = cload(be2r_d, [1, N])
            maskrow = cload(maskrow_d, [1, B * N])
            cvec = cload(cvec_d, [128, 3])

            # ---- distances: r2[y, x] then r = exp(0.5*ln(r2+eps)) ----
            r2ps = ps_sm.tile([N, XH], F32, name="r2ps", tag="sm")
            nc.tensor.matmul(r2ps[:], geoYL[:], geoXR[:], start=True, stop=True)
            rmat = wp.tile([N, XH], F32, name="rmat")
            r2sb = wp.tile([N, XH], F32, name="r2sb")
            nc.vector.tensor_scalar(
                r2sb[:], r2ps[:], 0.0, None, op0=ALU.max)
            nc.scalar.activation(rmat[:], r2sb[:], AF.Ln, bias=cvec[:N, 0:1])
            nc.scalar.activation(rmat[:], rmat[:], AF.Exp, scale=0.5)
            # Newton step: r <- 0.5*(r + r2/r) kills the pwp ln/exp error
            rinv = wp.tile([N, XH], F32, name="rinv")
            nc.vector.reciprocal(rinv[:], rmat[:])
            nc.vector.tensor_tensor(rinv[:], rinv[:], r2sb[:], op=ALU.mult)
            nc.vector.tensor_tensor(rmat[:], rmat[:], rinv[:], op=ALU.add)
            nc.vector.tensor_scalar_mul(rmat[:], rmat[:], 0.5)

            # ---- basis (packed): per x-column chunk c -> ybuf[:, 10c:10c+10]
            ybuf = bigp.tile([128, XH * NB], F32, name="ybuf")
            for c in range(XH):
                sl = ybuf[:, c * NB:(c + 1) * NB]
                # x = (r - g)/STEP  computed as (g - r) * (-1/STEP)
                nc.vector.tensor_scalar(
                    sl, gridb[:], rmat[:, c:c + 1], -1.0 / STEP,
                    op0=ALU.subtract, op1=ALU.mult)
                # clamp to [-1, 1]
                nc.vector.tensor_scalar(
                    sl, sl, 1.0, -1.0, op0=ALU.min, op1=ALU.max)
            # u = sin(pi/2 * y) with args in [-pi/2, pi/2]; basisT = u^2
            nc.scalar.activation(
                ybuf[:], ybuf[:], AF.Sin, scale=math.pi / 2)
            nc.vector.tensor_tensor(ybuf[:], ybuf[:], ybuf[:], op=ALU.mult)

            # transposes -> basisT [10, pairs] f32r (+1 folded into sigma bias)
            basisT = bigp.tile([NB, NP], F32R, name="basisT")
            for c in range(XH):
                tp = ps_sm.tile([NB, 128], F32, name="tpps", tag="sm")
                nc.tensor.transpose(tp[:], ybuf[:, c * NB:(c + 1) * NB], ident[:])
                nc.vector.tensor_copy(basisT[:, c * 128:(c + 1) * 128], tp[:])

            # gate mask broadcast [MUL, XH]
            mbps = ps_sm.tile([MUL, XH], F32, name="mbps", tag="sm")
            nc.tensor.matmul(mbps[:], ones1[:, 0:MUL], maskxr[:],
                             start=True, stop=True)
            mask_b32 = cp.tile([MUL, XH], F32, name="mask_b32")
            nc.vector.tensor_copy(mask_b32[:], mbps[:])

            # ---- fm prep helper: fsrc [N, EMB] -> fmT [EMB, N] (g_mmdt), fsum
            def fm_prep(fsrc, l):
                fmxi = wp.tile([N, EMB], F32, name="fmxi")
                nc.vector.tensor_scalar(
                    fmxi[:], fsrc[:], maskcol[:, 0:1], SQN,
                    op0=ALU.mult, op1=ALU.mult)
                fps = ps_sm.tile([EMB, N], F32, name="fmtps", tag="sm")
                nc.tensor.transpose(fps[:], fmxi[:], ident[:])
                fmT = wp.tile([EMB, N], g_mmdt, name="fmT")
                nc.vector.tensor_copy(fmT[:], fps[:])
                fsum = wp.tile([EMB, 1], F32, name="fsum")
                nc.vector.reduce_sum(fsum[:], fmT[:], axis=mybir.AxisListType.X)
                return fmT, fsum

            # ---- radial stack stage helpers ----
            CH = 512
            NCH = NP // CH  # 16 chunks per layer

            def mm1_sigma1(l, slot, after=None):
                sigs = []
                for ch in range(NCH):
                    mps = ps_mm.tile([HID, CH], F32, name="mmps")
                    off = ch * CH
                    nc.tensor.matmul(
                        mps[:], w1f[:, l * HID:(l + 1) * HID],
                        basisT[:, off:off + CH], start=True, stop=True)
                    si = nc.scalar.activation(
                        slot[:, off:off + CH], mps[:], AF.Sigmoid,
                        bias=b1f[:, l:l + 1], scale=-5.0)
                    if after is not None and ch == 0:
                        add_dep_helper(si.ins, after.ins,
                                       reason="batch act tables")
                    sigs.append(si)
                return sigs

            def mm2_sigma2(l, slot, after=None):
                # in-slot: mm2 reads s1' columns, sigma2 overwrites them
                sigs = []
                for ch in range(NCH):
                    mps = ps_mm.tile([HID, CH], F32, name="mmps")
                    off = ch * CH
                    nc.tensor.matmul(
                        mps[:], w2f[:, l * HID:(l + 1) * HID],
                        slot[:, off:off + CH], start=True, stop=True)
                    si = nc.scalar.activation(
                        slot[:, off:off + CH], mps[:], AF.Sigmoid,
                        bias=b2f[:, l:l + 1], scale=-5.0)
                    if after is not None and ch == 0:
                        add_dep_helper(si.ins, after.ins,
                                       reason="batch act tables")
                    sigs.append(si)
                return sigs

            def ln_inplace(slot):
                lns = []
                for h in range(2):
                    lns.append(nc.scalar.activation(
                        slot[:, h * (NP // 2):(h + 1) * (NP // 2)],
                        slot[:, h * (NP // 2):(h + 1) * (NP // 2)], AF.Ln))
                return lns

            # ---- f-chain for layer l: consumes s2b(l), fm(l); produces fnext
            agouts = []

            def f_chain(l, fmT, fsum, s2b):
                # G-stage: G[h, (i, y)] via 32 accumul.-free matmuls over j
                wg = gwp.tile([MUL, MUL * HID], g_mmdt, name="wgsb")
                nc.sync.dma_start(wg[:], wg_d[:, l * MUL * HID:(l + 1) * MUL * HID])
                gbuf = gwp.tile([HID, MUL * N], G_DT, name="gbuf")
                for i4 in range(MUL // 4):
                    gps = ps_g.tile([HID, 4 * N], F32, name="gps", tag="g")
                    for k in range(4):
                        i = i4 * 4 + k
                        nc.tensor.matmul(
                            gps[:, k * N:(k + 1) * N],
                            wg[:, i * HID:(i + 1) * HID],
                            fmT[:], start=True, stop=True)
                    nc.vector.tensor_copy(
                        gbuf[:, i4 * 4 * N:(i4 + 1) * 4 * N], gps[:])
                # constant row: drow[i, 1] = w3c_l^T @ fsum (per-partition)
                cps = ps_sm.tile([MUL, 1], F32, name="cps", tag="sm")
                nc.tensor.matmul(
                    cps[:], w3c[:, l * MUL:(l + 1) * MUL], fsum[:],
                    start=True, stop=True)
                drow = wp.tile([MUL, 1], F32, name="drow")
                nc.vector.tensor_copy(drow[:], cps[:])

                # final contraction: psum_f[i, x] = sum_y G_y^T s2'_y
                fps = ps_sm.tile([MUL, XH], F32, name="fckps", tag="sm")
                for y in range(N):
                    nc.tensor.matmul(
                        fps[:],
                        gbuf[:, y::N],      # [HID, MUL] strided (i outer, y inner)
                        s2b[:, y::N],       # [HID, XH] strided (x outer, y inner)
                        start=(y == 0), stop=(y == N - 1))

                # gate: fnext = softplus(5*(t+drow))/5 * mask
                #   = (relu(t') + 0.2*ln(1+exp(-5*|t'|))) * mask
                tval = wp.tile([MUL, XH], F32, name="gt_t")
                nc.vector.tensor_scalar(
                    tval[:], fps[:], drow[:, 0:1], None, op0=ALU.add)
                tneg = wp.tile([MUL, XH], F32, name="gt_n")
                nc.vector.tensor_scalar_mul(tneg[:], tval[:], -1.0)
                tabs = wp.tile([MUL, XH], F32, name="gt_a")
                nc.vector.tensor_tensor(tabs[:], tval[:], tneg[:], op=ALU.max)
                gexp = wp.tile([MUL, XH], F32, name="gt_e")
                nc.scalar.activation(gexp[:], tabs[:], AF.Exp, scale=-5.0)
                nc.scalar.activation(gexp[:], gexp[:], AF.Ln, bias=nc.const_aps.scalar_like(1.0, gexp[:]))
                relu_t = wp.tile([MUL, XH], F32, name="gt_r")
                nc.vector.tensor_scalar(
                    relu_t[:], tval[:], 0.0, None, op0=ALU.max)
                fnext32 = wp.tile([MUL, XH], F32, name="fnext32")
                nc.vector.tensor_scalar(
                    fnext32[:], gexp[:], 0.2, None, op0=ALU.mult)
                nc.vector.tensor_tensor(
                    fnext32[:], fnext32[:], relu_t[:], op=ALU.add)
                nc.vector.tensor_tensor(
                    fnext32[:], fnext32[:], mask_b32[:], op=ALU.mult)
                # transpose to [XH, MUL] for the partition-concat AllGather
                ftp = ps_sm.tile([XH, MUL], F32, name="ftp", tag="sm")
                nc.tensor.transpose(ftp[:], fnext32[:], ident[0:MUL, 0:MUL])
                fnext = wp.tile([XH, MUL], F32, name="fnext")
                nc.vector.tensor_copy(fnext[:], ftp[:])

                # AllGather
                agi = dp.tile([XH, MUL], F32, name=f"agi{l}")
                nc.sync.dma_start(agi[:], fnext[:])
                if l < NL - 1:
                    ago = dp.tile([N, MUL], F32, name=f"ago{l}")
                    nc.gpsimd.collective_compute(
                        "AllGather", ALU.bypass,
                        replica_groups=[[0, 1], [2, 3], [4, 5], [6, 7]],
                        ins=[agi.opt()], outs=[ago.opt()])
                else:
                    ago = dp.tile([B * N, MUL], F32, name="agofin")
                    nc.gpsimd.collective_compute(
                        "AllGather", ALU.bypass,
                        replica_groups=[list(range(NCORES))],
                        ins=[agi.opt()], outs=[ago.opt()])
                agouts.append(ago)
                return ago

            # ================= schedule =================
            fmT0, fsum0 = fm_prep(f0sb, 0)

            S = [None] * NL
            fm = [(fmT0, fsum0)] + [None] * (NL - 1)

            prev_ln2_last = None
            for wave in range(2):
                l0, l1 = 2 * wave, 2 * wave + 1
                S[l0] = slotp.tile([HID, NP], F32R, name=f"slot{l0}", tag="slot")
                S[l1] = slotp.tile([HID, NP], F32R, name=f"slot{l1}", tag="slot")
                mm1_sigma1(l0, S[l0], after=prev_ln2_last)
                mm1_sigma1(l1, S[l1])
                ln1a = ln_inplace(S[l0])
                ln1b = ln_inplace(S[l1])
                mm2_sigma2(l0, S[l0], after=ln1b[-1])
                mm2_sigma2(l1, S[l1])
                s2w = [None, None]
                ln2_last = None
                for k, l in enumerate((l0, l1)):
                    s2w[k] = s2wp.tile([HID, NP], BF16, name=f"s2w{l}", tag="s2w")
                    for h in range(2):
                        ln2_last = nc.scalar.activation(
                            s2w[k][:, h * (NP // 2):(h + 1) * (NP // 2)],
                            S[l][:, h * (NP // 2):(h + 1) * (NP // 2)], AF.Ln)
                prev_ln2_last = ln2_last
                # f-chains consume s2w (bf16)
                for k, l in enumerate((l0, l1)):
                    fmT_l, fsum_l = fm[l]
                    ago = f_chain(l, fmT_l, fsum_l, s2w[k])
                    if l < NL - 1:
                        fsrc = wp.tile([N, EMB], F32, name="fsrc")
                        nc.sync.dma_start(fsrc[:], ago.opt())
                        fm[l + 1] = fm_prep(fsrc, l + 1)

            # ================= final MLP (redundant on all cores) ========
            fall = wp.tile([128, B * EMB], F32, name="fall")
            agofin = agouts[-1]
            nc.sync.dma_start(
                fall[:].rearrange("x (b j) -> x b j", b=B),
                agofin.opt().rearrange("(b x) j -> x b j", b=B))
            fT = wp.tile([EMB, B * N], F32, name="fTall")
            for b in range(B):
                tps = ps_sm.tile([EMB, 128], F32, name="ftps", tag="sm")
                nc.tensor.transpose(
                    tps[:], fall[:, b * EMB:(b + 1) * EMB], ident[:])
                nc.vector.tensor_copy(fT[:, b * N:(b + 1) * N], tps[:])

            def bn_layer(xT, nchunk, wl, bl, gr, ber):
                # xT: [EMB or 128, B*N] rhs; wl lhsT chunks [K, 128]*nchunk
                a_s = []
                for m in range(nchunk):
                    aps = ps_g.tile([128, B * N], F32, name="aps", tag="g")
                    nc.tensor.matmul(
                        aps[:], wl[:, m * 128:(m + 1) * 128], xT[:],
                        start=True, stop=True)
                    asb = wp.tile([128, B * N], F32, name="asb", tag="asb", bufs=3)
                    nc.vector.tensor_scalar_add(asb[:], aps[:], bl[:, m:m + 1])
                    a_s.append(asb)
                # stats over (b, channel) per atom x
                sps = ps_sm.tile([1, B * N], F32, name="sps", tag="sm")
                qps = ps_sm.tile([1, B * N], F32, name="qps", tag="sm")
                for m in range(nchunk):
                    nc.tensor.matmul(
                        sps[:], ones128[:], a_s[m][:],
                        start=(m == 0), stop=(m == nchunk - 1))
                sqs = []
                for m in range(nchunk):
                    sq = wp.tile([128, B * N], F32, name="sq", tag="sq")
                    nc.vector.tensor_tensor(
                        sq[:], a_s[m][:], a_s[m][:], op=ALU.mult)
                    sqs.append(sq)
                for m in range(nchunk):
                    nc.tensor.matmul(
                        qps[:], ones128[:], sqs[m][:],
                        start=(m == 0), stop=(m == nchunk - 1))
                # fold batch: [1, B*N] -> [1, N]
                ssb = wp.tile([1, B * N], F32, name="ssb", tag="row512", bufs=4)
                qsb = wp.tile([1, B * N], F32, name="qsb", tag="row512", bufs=4)
                nc.vector.tensor_copy(ssb[:], sps[:])
                nc.vector.tensor_copy(qsb[:], qps[:])
                mu = wp.tile([1, N], F32, name="mu", tag="row128", bufs=6)
                var = wp.tile([1, N], F32, name="var", tag="row128", bufs=6)
                nc.vector.tensor_tensor(
                    mu[:], ssb[:, 0:N], ssb[:, N:2 * N], op=ALU.add)
                nc.vector.tensor_tensor(
                    mu[:], mu[:], ssb[:, 2 * N:3 * N], op=ALU.add)
                nc.vector.tensor_tensor(
                    mu[:], mu[:], ssb[:, 3 * N:4 * N], op=ALU.add)
                nc.vector.tensor_tensor(
                    var[:], qsb[:, 0:N], qsb[:, N:2 * N], op=ALU.add)
                nc.vector.tensor_tensor(
                    var[:], var[:], qsb[:, 2 * N:3 * N], op=ALU.add)
                nc.vector.tensor_tensor(
                    var[:], var[:], qsb[:, 3 * N:4 * N], op=ALU.add)
                cnt = float(B * 128 * nchunk)
                nc.vector.tensor_scalar_mul(mu[:], mu[:], 1.0 / cnt)
                nc.vector.tensor_scalar_mul(var[:], var[:], 1.0 / cnt)
                musq = wp.tile([1, N], F32, name="musq", tag="row128", bufs=6)
                nc.vector.tensor_tensor(musq[:], mu[:], mu[:], op=ALU.mult)
                nc.vector.tensor_tensor(var[:], var[:], musq[:], op=ALU.subtract)
                # inv = exp(-0.5*ln(var+eps)); s = g*inv; t = be - mu*s
                inv = wp.tile([1, N], F32, name="inv", tag="row128", bufs=6)
                nc.scalar.activation(inv[:], var[:], AF.Ln, bias=cvec[:1, 2:3])
                nc.scalar.activation(inv[:], inv[:], AF.Exp, scale=-0.5)
                svec = wp.tile([1, N], F32, name="svec", tag="row128", bufs=6)
                nc.vector.tensor_tensor(svec[:], gr[:], inv[:], op=ALU.mult)
                tvec = wp.tile([1, N], F32, name="tvec", tag="row128", bufs=6)
                nc.vector.tensor_tensor(tvec[:], mu[:], svec[:], op=ALU.mult)
                nc.vector.tensor_scalar_mul(tvec[:], tvec[:], -1.0)
                nc.vector.tensor_tensor(tvec[:], ber[:], tvec[:], op=ALU.add)
                # widen to [1, B*N] then broadcast via rank-1 matmul
                sw = wp.tile([1, B * N], F32, name="sw", tag="row512", bufs=4)
                tw = wp.tile([1, B * N], F32, name="tw", tag="row512", bufs=4)
                for b in range(B):
                    nc.vector.tensor_copy(sw[:, b * N:(b + 1) * N], svec[:])
                    nc.vector.tensor_copy(tw[:, b * N:(b + 1) * N], tvec[:])
                sB = ps_g.tile([128, B * N], F32, name="sB", tag="g")
                tB = ps_g.tile([128, B * N], F32, name="tB", tag="g")
                nc.tensor.matmul(sB[:], ones1[:], sw[:], start=True, stop=True)
                nc.tensor.matmul(tB[:], ones1[:], tw[:], start=True, stop=True)
                outs = []
                for m in range(nchunk):
                    nc.vector.tensor_tensor(
                        a_s[m][:], a_s[m][:], sB[:], op=ALU.mult)
                    nc.vector.tensor_tensor(
                        a_s[m][:], a_s[m][:], tB[:], op=ALU.add)
                    nc.scalar.activation(
                        a_s[m][:], a_s[m][:], AF.Prelu, alpha=0.2)
                    outs.append(a_s[m])
                return outs

            h1 = bn_layer(fT, 2, w1c, b1c, g1r, be1r)
            # pack h1 chunks for layer 2: rhs must be [128, B*N] per chunk
            aps2 = ps_g.tile([128, B * N], F32, name="aps2", tag="g")
            for k in range(2):
                nc.tensor.matmul(
                    aps2[:], w2c[:, k * 128:(k + 1) * 128], h1[k][:],
                    start=(k == 0), stop=(k == 1))
            h2sb = wp.tile([128, B * N], F32, name="h2sb", tag="asb", bufs=3)
            nc.vector.tensor_scalar_add(h2sb[:], aps2[:], b2c[:, 0:1])
            # BN2 (single chunk of 128 channels)
            sps2 = ps_sm.tile([1, B * N], F32, name="sps2", tag="sm")
            qps2 = ps_sm.tile([1, B * N], F32, name="qps2", tag="sm")
            nc.tensor.matmul(sps2[:], ones128[:], h2sb[:], start=True, stop=True)
            sq2 = wp.tile([128, B * N], F32, name="sq2", tag="sq")
            nc.vector.tensor_tensor(sq2[:], h2sb[:], h2sb[:], op=ALU.mult)
            nc.tensor.matmul(qps2[:], ones128[:], sq2[:], start=True, stop=True)
            ssb2 = wp.tile([1, B * N], F32, name="ssb2", tag="row512", bufs=4)
            qsb2 = wp.tile([1, B * N], F32, name="qsb2", tag="row512", bufs=4)
            nc.vector.tensor_copy(ssb2[:], sps2[:])
            nc.vector.tensor_copy(qsb2[:], qps2[:])
            mu2 = wp.tile([1, N], F32, name="mu2", tag="row128", bufs=6)
            var2 = wp.tile([1, N], F32, name="var2", tag="row128", bufs=6)
            nc.vector.tensor_tensor(mu2[:], ssb2[:, 0:N], ssb2[:, N:2 * N], op=ALU.add)
            nc.vector.tensor_tensor(mu2[:], mu2[:], ssb2[:, 2 * N:3 * N], op=ALU.add)
            nc.vector.tensor_tensor(mu2[:], mu2[:], ssb2[:, 3 * N:4 * N], op=ALU.add)
            nc.vector.tensor_tensor(var2[:], qsb2[:, 0:N], qsb2[:, N:2 * N], op=ALU.add)
            nc.vector.tensor_tensor(var2[:], var2[:], qsb2[:, 2 * N:3 * N], op=ALU.add)
            nc.vector.tensor_tensor(var2[:], var2[:], qsb2[:, 3 * N:4 * N], op=ALU.add)
            cnt2 = float(B * 128)
            nc.vector.tensor_scalar_mul(mu2[:], mu2[:], 1.0 / cnt2)
            nc.vector.tensor_scalar_mul(var2[:], var2[:], 1.0 / cnt2)
            musq2 = wp.tile([1, N], F32, name="musq2", tag="row128", bufs=6)
            nc.vector.tensor_tensor(musq2[:], mu2[:], mu2[:], op=ALU.mult)
            nc.vector.tensor_tensor(var2[:], var2[:], musq2[:], op=ALU.subtract)
            inv2 = wp.tile([1, N], F32, name="inv2", tag="row128", bufs=6)
            nc.scalar.activation(inv2[:], var2[:], AF.Ln, bias=cvec[:1, 2:3])
            nc.scalar.activation(inv2[:], inv2[:], AF.Exp, scale=-0.5)
            svec2 = wp.tile([1, N], F32, name="svec2", tag="row128", bufs=6)
            nc.vector.tensor_tensor(svec2[:], g2r[:], inv2[:], op=ALU.mult)
            tvec2 = wp.tile([1, N], F32, name="tvec2", tag="row128", bufs=6)
            nc.vector.tensor_tensor(tvec2[:], mu2[:], svec2[:], op=ALU.mult)
            nc.vector.tensor_scalar_mul(tvec2[:], tvec2[:], -1.0)
            nc.vector.tensor_tensor(tvec2[:], be2r[:], tvec2[:], op=ALU.add)
            sw2 = wp.tile([1, B * N], F32, name="sw2", tag="row512", bufs=4)
            tw2 = wp.tile([1, B * N], F32, name="tw2", tag="row512", bufs=4)
            for b in range(B):
                nc.vector.tensor_copy(sw2[:, b * N:(b + 1) * N], svec2[:])
                nc.vector.tensor_copy(tw2[:, b * N:(b + 1) * N], tvec2[:])
            sB2 = ps_g.tile([128, B * N], F32, name="sB2", tag="g")
            tB2 = ps_g.tile([128, B * N], F32, name="tB2", tag="g")
            nc.tensor.matmul(sB2[:], ones1[:], sw2[:], start=True, stop=True)
            nc.tensor.matmul(tB2[:], ones1[:], tw2[:], start=True, stop=True)
            nc.vector.tensor_tensor(h2sb[:], h2sb[:], sB2[:], op=ALU.mult)
            nc.vector.tensor_tensor(h2sb[:], h2sb[:], tB2[:], op=ALU.add)
            nc.scalar.activation(h2sb[:], h2sb[:], AF.Prelu, alpha=0.2)

            # masked mean pool over atoms -> pooledT [128 o, B]
            mB = ps_g.tile([128, B * N], F32, name="mB", tag="g")
            nc.tensor.matmul(mB[:], ones1[:], maskrow[:], start=True, stop=True)
            nc.vector.tensor_tensor(h2sb[:], h2sb[:], mB[:], op=ALU.mult)
            psum_pool = wp.tile([128, B], F32, name="psum_pool")
            nc.vector.reduce_sum(
                psum_pool[:],
                h2sb[:].rearrange("p (b x) -> p b x", b=B),
                axis=mybir.AxisListType.X)
            msum = wp.tile([1, B], F32, name="msum")
            nc.vector.reduce_sum(
                msum[:],
                maskrow[:].rearrange("p (b x) -> p b x", b=B),
                axis=mybir.AxisListType.X)
            minv = wp.tile([1, B], F32, name="minv")
            nc.vector.reciprocal(minv[:], msum[:])
            mvB = ps_sm.tile([128, B], F32, name="mvB", tag="sm")
            nc.tensor.matmul(mvB[:], ones1[:], minv[:], start=True, stop=True)
            nc.vector.tensor_tensor(
                psum_pool[:], psum_pool[:], mvB[:], op=ALU.mult)

            # out[b, o] <- psum_pool[o, b]
            nc.sync.dma_start(
                out_d[:].rearrange("b o -> o b"), psum_pool[:])

    nc.compile()
    return nc


def _host_prep(inputs):
    """Fold all affine constants/signs into weights; build per-core in_maps."""
    f = {k: np.asarray(v) for k, v in inputs.items()}
    geometry = f["geometry"].astype(np.float64)
    features = f["features"].astype(np.int64)
    mask = f["mask"].astype(np.float64)
    emb = f["emb"].astype(np.float64)
    rw1, rw2, rw3 = (f[k].astype(np.float64) for k in ("rw1", "rw2", "rw3"))
    W1, b1 = f["W1"].astype(np.float64), f["b1"].astype(np.float64)
    W2, b2 = f["W2"].astype(np.float64), f["b2"].astype(np.float64)
    g1, be1 = f["g1"].astype(np.float64), f["be1"].astype(np.float64)
    g2, be2 = f["g2"].astype(np.float64), f["be2"].astype(np.float64)

    f32 = np.float32
    # radial weights folding
    w1f = np.zeros((NB, NL * HID))
    b1f = np.zeros((HID, NL))
    w2f = np.zeros((HID, NL * HID))
    b2f = np.zeros((HID, NL))
    wg = np.zeros((MUL, NL * MUL * HID))
    w3c = np.zeros((MUL, NL * MUL))
    for l in range(NL):
        w1n = rw1[l] / math.sqrt(NB)                # [10, 128]
        w1f[:, l * HID:(l + 1) * HID] = -w1n
        b1f[:, l] = -5.0 * w1n.sum(axis=0)
        w2l = -rw2[l] / (5.0 * math.sqrt(HID))      # [128, 128]
        w2f[:, l * HID:(l + 1) * HID] = w2l
        b2f[:, l] = -5.0 * LN2 * w2l.sum(axis=0)
        # wg[l][i][j, h] = -(Y0/(5*sqrt(HID))) * rw3[l][h, i*MUL+j]
        r3 = rw3[l].reshape(HID, MUL, MUL)          # [h, i, j]
        gfac = -(Y0 / (5.0 * math.sqrt(HID)))
        wgl = gfac * r3.transpose(2, 1, 0)          # [j, i, h]
        wg[:, l * MUL * HID:(l + 1) * MUL * HID] = wgl.reshape(MUL, MUL * HID)
        # w3c[l][j, i] = -(LN2*Y0/(5*sqrt(HID))) * sum_h rw3[l][h, i*32+j]
        w3cl = -(LN2 * Y0 / (5.0 * math.sqrt(HID))) * r3.sum(axis=0).T  # [j, i]
        w3c[:, l * MUL:(l + 1) * MUL] = w3cl

    # final MLP packing
    w1c = W1                                         # [32, 256]
    b1c = b1.reshape(2, 128).T                       # [128, 2]
    w2c = np.zeros((128, MID))
    for k in range(2):
        w2c[:, k * 128:(k + 1) * 128] = W2[k * 128:(k + 1) * 128, :]
    b2c = b2.reshape(128, 1)

    grid = np.linspace(0.0, MAXR, NB)
    gridb = np.tile(grid[None, :], (128, 1))

    shared = {
        "gridb": gridb.astype(f32),
        "ident": np.eye(128, dtype=f32),
        "ones1": np.ones((1, 128), f32),
        "ones128": np.ones((128, 1), f32),
        "w1f": w1f.astype(f32), "b1f": b1f.astype(f32),
        "w2f": w2f.astype(f32), "b2f": b2f.astype(f32),
        "w3c": w3c.astype(f32),
        "w1c": w1c.astype(f32), "b1c": b1c.astype(f32),
        "w2c": w2c.astype(f32), "b2c": b2c.astype(f32),
        "g1r": g1.reshape(1, N).astype(f32),
        "be1r": be1.reshape(1, N).astype(f32),
        "g2r": g2.reshape(1, N).astype(f32),
        "be2r": be2.reshape(1, N).astype(f32),
        "maskrow": mask.reshape(1, B * N).astype(f32),
        "cvec": np.tile(np.array([1e-12, math.pi / 2, 1e-5], np.float32),
                        (128, 1)),
    }
    import ml_dtypes
    shared["wg"] = wg.astype(ml_dtypes.bfloat16)

    f0_all = emb[features[..., 0]]                   # [B, N, EMB]
    norms = (geometry ** 2).sum(axis=-1)             # [B, N]

    in_maps = []
    for c in range(NCORES):
        b = c // 2
        x0 = (c % 2) * XH
        geoYL = np.zeros((5, N))
        geoYL[0:3] = -2.0 * geometry[b].T
        geoYL[3] = norms[b]
        geoYL[4] = 1.0
        geoXR = np.zeros((5, XH))
        geoXR[0:3] = geometry[b, x0:x0 + XH].T
        geoXR[3] = 1.0
        geoXR[4] = norms[b, x0:x0 + XH]
        m = dict(shared)
        m["geoYL"] = geoYL.astype(f32)
        m["geoXR"] = geoXR.astype(f32)
        m["f0"] = f0_all[b].astype(f32)
        m["maskcol"] = mask[b].reshape(N, 1).astype(f32)
        m["maskxr"] = mask[b, x0:x0 + XH].reshape(1, XH).astype(f32)
        in_maps.append(m)
    return in_maps


def run(inputs, trace=False):
    global _cached
    from concourse import bass_utils
    if _cached is None:
        _cached = _build()
    nc = _cached
    in_maps = _host_prep(inputs)
    res = bass_utils.run_bass_kernel_spmd(
        nc, in_maps, core_ids=list(range(NCORES)), trace=trace)
    return res


def kernel(**inputs):
    res = run(inputs, trace=False)
    return np.asarray(res.results[0]["out"], dtype=np.float32)



# revision 10
# speedup vs baseline: 1.8301x; 1.8301x over previous
"""Bass/Tile TRN2 kernel for nn_Network_21131239096982 (gnn_message_passing).

Sharding: 8 cores = 4 samples x 2 y-halves (full x per core). Pair order
(y outer, x inner). Per-layer ReduceScatter(cc_dim=Free) over the pair
sums the y-half partial preactivations and hands each core exactly its
own y-half of the next layer's features (rank0 -> cols 0:64, rank1 ->
cols 64:128), so the SPMD program needs no per-core offsets. Layer 3
does an 8-way AllGather of the [32, 128] partials; each core sums the
pair blocks and runs the batchnorm MLP head redundantly.

Key restructure vs the reference: the radial MLP (cosine basis -> ssp
-> ssp -> w3) is 128 univariate functions of r, host-tabulated on a
128-point r-uniform grid and evaluated on device as ONE matmul per pair
chunk:  s2[h, pair] = Ftab_l[g, h]^T @ O[g, pair],  where O holds
linear-interpolation hat weights in u = r^2 (asymmetric triangular
hats -> no sqrt needed). O is built once from a rank-1 PE broadcast
psum = sl_g * u plus three DVE ops per chunk. The gate
softplus(5t)/5 = relu(t) + poly9(min(|t|,2)) runs on DVE (+ Abs/Relu,
present in every ACT table set), so the only ACT table load in the
whole kernel is ln/exp for the two batchnorm inverse-stddevs.
"""

import math

import numpy as np

B, N, EMB, MUL = 4, 128, 32, 32
NB, MAXR = 10, 10.0
HID, BETA = 128, 5.0
MID, OUT = 256, 128
NL = 4
Y0 = 1.0 / (2.0 * math.sqrt(math.pi))
YH = N // 2          # 64 local y's per core
NP = N * YH          # 8192 pairs per core, order (y outer, x inner)
NCORES = 8
GRID_N = 128
RMAX = 7.5
SQN = 1.0 / math.sqrt(N)

# softplus(5t)/5 - relu(t) = ln(1+exp(-5|t|))/5, chebyshev fit on [0, 2]
GATE_PC = [0.13863592819866152, -0.4999284878393997, 0.6156649023363564,
           0.12337920499527943, -1.3216523110767724, 1.8311453040108088,
           -1.3266478452560657, 0.5557922376483523, -0.1274729154222193,
           0.012418893315223408]

_cached = None


def _build():
    import jax

    jax.devices()  # axon boot
    from concourse import bacc, tile, mybir

    F32 = mybir.dt.float32
    BF16 = mybir.dt.bfloat16
    AF = mybir.ActivationFunctionType
    ALU = mybir.AluOpType

    nc = bacc.Bacc("TRN2", debug=False, num_devices=NCORES)

    def din(name, shape, dt=F32):
        return nc.dram_tensor(name, shape, dt, kind="ExternalInput").ap()

    geoY_d = din("geoY", [5, YH])
    geoX_d = din("geoX", [5, N])
    slrow_d = din("slrow", [1, GRID_N], BF16)
    coef_d = din("coef", [GRID_N, 3])          # A-add, B-mult, B-add
    ftab_d = din("ftab", [GRID_N, NL * HID], BF16)
    wg_d = din("wg", [MUL, NL * MUL * HID], BF16)
    fm0_d = din("fm0", [MUL, YH], BF16)        # my y-half of fm layer 0
    msqn_d = din("msqn", [YH, MUL])            # mask_half/sqrt(N) x ones32
    ident64_d = din("ident64", [YH, YH])
    mhead_d = din("mhead", [MUL, B * N])       # ones32 x mask blocks
    w1c_d = din("w1c", [EMB, MID], BF16)
    b1c_d = din("b1c", [128, 2])
    w2c_d = din("w2c", [128, MID], BF16)
    b2c_d = din("b2c", [128, 1])
    gb1_d = din("gb1", [1, 2 * N])             # g1r | be1r
    gb2_d = din("gb2", [1, 2 * N])             # g2r | be2r
    ones128_d = din("ones128", [128, 1], BF16)
    ones1_d = din("ones1", [1, 128])
    maskB_d = din("maskB", [128, B * N], BF16)
    mvB_d = din("mvB", [128, B])
    epsv_d = din("epsv", [1, 1])
    out_d = nc.dram_tensor("out", [B, OUT], F32, kind="ExternalOutput").ap()

    UMAX = (RMAX ** 2) * (1.0 - 1e-4)
    CH = 1024            # pair columns per psum tile (2 matmuls of 512)
    NCH = NP // CH       # 8

    with tile.TileContext(nc) as tc:
        with (
            tc.tile_pool(name="const", bufs=1) as cp,
            tc.tile_pool(name="s2p", bufs=2) as s2p,
            tc.tile_pool(name="gbp", bufs=2) as gbp,
            tc.tile_pool(name="fmp", bufs=2) as fmp,
            tc.tile_pool(name="wk", bufs=2) as wk,
            tc.tile_pool(name="hd", bufs=2) as hd,
            tc.tile_pool(name="ps_big", bufs=2, space="PSUM") as pA,
            tc.tile_pool(name="ps_g", bufs=2, space="PSUM") as pG,
            tc.tile_pool(name="ps_fc", bufs=2, space="PSUM") as pF,
            tc.tile_pool(name="dram", bufs=1, space="DRAM") as dp,
        ):
            def cload(ap, shape, dt=F32):
                t = cp.tile(shape, dt, name=ap.tensor.name + "_sb")
                nc.sync.dma_start(t[:], ap[:])
                return t

            geoY = cload(geoY_d, [5, YH])
            geoX = cload(geoX_d, [5, N])
            slrow = cload(slrow_d, [1, GRID_N], BF16)
            coef = cload(coef_d, [GRID_N, 3])
            fm0 = cload(fm0_d, [MUL, YH], BF16)
            msqn = cload(msqn_d, [YH, MUL])
            ident64 = cload(ident64_d, [YH, YH])
            ftab = cload(ftab_d, [GRID_N, NL * HID], BF16)
            wg = cload(wg_d, [MUL, NL * MUL * HID], BF16)
            mhead = cload(mhead_d, [MUL, B * N])
            w1c = cload(w1c_d, [EMB, MID], BF16)
            b1c = cload(b1c_d, [128, 2])
            w2c = cload(w2c_d, [128, MID], BF16)
            b2c = cload(b2c_d, [128, 1])
            gb1 = cload(gb1_d, [1, 2 * N])
            gb2 = cload(gb2_d, [1, 2 * N])
            ones128 = cload(ones128_d, [128, 1], BF16)
            ones1 = cload(ones1_d, [1, 128])
            maskB = cload(maskB_d, [128, B * N], BF16)
            mvB = cload(mvB_d, [128, B])
            epsv = cload(epsv_d, [1, 1])

            # ---- u = r^2 [y, x] clamped bf16; flatten via DRAM bounce ----
            r2ps = pG.tile([YH, N], F32, name="r2ps", tag="g")
            nc.tensor.matmul(r2ps[:], geoY[:], geoX[:], start=True, stop=True)
            u2d = wk.tile([YH, N], BF16, name="u2d", tag="u2d")
            nc.vector.tensor_scalar(
                u2d[:], r2ps[:], 0.0, UMAX, op0=ALU.max, op1=ALU.min)
            ubounce = dp.tile([YH, N], BF16, name="ubounce")
            nc.sync.dma_start(ubounce[:], u2d[:])
            urow = cp.tile([1, NP], BF16, name="urow")
            nc.sync.dma_start(
                urow[:], ubounce.opt().rearrange("p x -> () (p x)"))

            # ---- O[g, pair]: linear-interp hats in u ----
            # psum = sl_g*u ; A = psum + coef0 ; B = psum*coef1 + coef2
            # O = relu(min(A, B))
            obuf = cp.tile([GRID_N, NP], BF16, name="obuf")
            for c in range(NCH):
                ups = pA.tile([GRID_N, CH], F32, name="ups", tag="big")
                for h in range(2):
                    nc.tensor.matmul(
                        ups[:, h * 512:(h + 1) * 512], slrow[:],
                        urow[:, c * CH + h * 512:c * CH + (h + 1) * 512],
                        start=True, stop=True)
                osl = obuf[:, c * CH:(c + 1) * CH]
                bt = wk.tile([GRID_N, CH], F32, name="btile", tag="btile")
                nc.vector.tensor_scalar(
                    bt[:], ups[:], coef[:, 1:2], coef[:, 2:3],
                    op0=ALU.mult, op1=ALU.add)
                nc.vector.scalar_tensor_tensor(
                    osl, ups[:], coef[:, 0:1], bt[:],
                    op0=ALU.add, op1=ALU.min)
                nc.vector.tensor_scalar(osl, osl, 0.0, None, op0=ALU.max)

            # ---- gate helper ----
            def gate_chain(src_ap, pdim, width, mask_ap, name, res_dt=BF16):
                tt = wk.tile([pdim, width], F32, name=f"tt{name}", tag="gt", bufs=6)
                nc.scalar.activation(tt[:], src_ap, AF.Abs)
                nc.vector.tensor_scalar(tt[:], tt[:], 2.0, None, op0=ALU.min)
                # recurrence q=(q+c)*u gives a9*u^9+(c1)u^8+...+(c8)u,
                # so feed c_j = a_{9-j}; a0 folds into the final mask STT.
                pv = wk.tile([pdim, width], F32, name=f"pv{name}", tag="gt", bufs=6)
                nc.vector.tensor_scalar(
                    pv[:], tt[:], GATE_PC[9], None, op0=ALU.mult)
                for k in range(8, 0, -1):
                    nc.vector.scalar_tensor_tensor(
                        pv[:], pv[:], GATE_PC[k], tt[:],
                        op0=ALU.add, op1=ALU.mult)
                rl = wk.tile([pdim, width], F32, name=f"rl{name}", tag="gt", bufs=6)
                nc.scalar.activation(rl[:], src_ap, AF.Relu)
                nc.vector.tensor_tensor(pv[:], pv[:], rl[:], op=ALU.add)
                res = fmp.tile([pdim, width], res_dt, name=f"fm{name}", tag="fm")
                nc.vector.scalar_tensor_tensor(
                    res[:], pv[:], GATE_PC[0], mask_ap,
                    op0=ALU.add, op1=ALU.mult)
                return res

            # ---- conv layers ----
            fm = [fm0] + [None] * NL
            part3 = None
            for l in range(NL):
                # radial: s2_l[h, (y, x)] = Ftab_l^T @ O
                s2 = s2p.tile([HID, NP], BF16, name=f"s2_{l}", tag="s2")
                for c in range(NCH):
                    rps = pA.tile([HID, CH], F32, name="rps", tag="big")
                    for h in range(2):
                        nc.tensor.matmul(
                            rps[:, h * 512:(h + 1) * 512],
                            ftab[:, l * HID:(l + 1) * HID],
                            obuf[:, c * CH + h * 512:c * CH + (h + 1) * 512],
                            start=True, stop=True)
                    dst = s2[:, c * CH:(c + 1) * CH]
                    if c % 2 == 0:
                        nc.scalar.activation(dst, rps[:], AF.Copy)
                    else:
                        nc.vector.tensor_copy(dst, rps[:])

                # G-stage: gbuf[h, (i, y)] in blocks of 4 i's
                gbuf = gbp.tile([HID, MUL * YH], BF16, name=f"gb{l}", tag="gb")
                for q in range(MUL // 4):
                    gps = pG.tile([HID, 4 * YH], F32, name="gps", tag="g")
                    for k in range(4):
                        i = q * 4 + k
                        nc.tensor.matmul(
                            gps[:, k * YH:(k + 1) * YH],
                            wg[:, (l * MUL + i) * HID:(l * MUL + i + 1) * HID],
                            fm[l][:], start=True, stop=True)
                    nc.vector.tensor_copy(
                        gbuf[:, q * 4 * YH:(q + 1) * 4 * YH], gps[:])

                # final contraction over my y-half
                gview = gbuf[:].rearrange("p (i y) -> p y i", y=YH)
                if l < NL - 1:
                    # transposed partial [x, i]: flat RS halves == y-halves
                    pf = pF.tile([N, MUL], F32, name=f"pf{l}", tag="fc")
                    for y in range(YH):
                        nc.tensor.matmul(
                            pf[:], s2[:, y * N:(y + 1) * N], gview[:, y, :],
                            start=(y == 0), stop=(y == YH - 1))
                    part = wk.tile([N, MUL], F32, name=f"part{l}", tag="part")
                    nc.vector.tensor_copy(part[:], pf[:])
                    ari = dp.tile([N, MUL], F32, name=f"ari{l}")
                    nc.sync.dma_start(ari[:], part[:])
                    aro = dp.tile([YH, MUL], F32, name=f"aro{l}")
                    nc.gpsimd.collective_compute(
                        "ReduceScatter", ALU.add,
                        replica_groups=[[0, 1], [2, 3], [4, 5], [6, 7]],
                        ins=[ari.opt()], outs=[aro.opt()], cc_dim="Free")
                    pre = wk.tile([YH, MUL], F32, name=f"pre{l}", tag="pre")
                    nc.sync.dma_start(pre[:], aro.opt())
                    gfm = gate_chain(pre[:], YH, MUL, msqn[:], f"{l}", res_dt=F32)
                    # transpose [y, i] -> fm [i, y] for the next G-stage
                    ftp = pG.tile([MUL, YH], F32, name=f"ftp{l}", tag="g")
                    nc.tensor.transpose(ftp[:], gfm[:], ident64[:])
                    fmn = fmp.tile([MUL, YH], BF16, name=f"fmn{l}", tag="fm")
                    nc.vector.tensor_copy(fmn[:], ftp[:])
                    fm[l + 1] = fmn
                else:
                    pf = pF.tile([MUL, N], F32, name=f"pf{l}", tag="fc")
                    for y in range(YH):
                        nc.tensor.matmul(
                            pf[:], gview[:, y, :], s2[:, y * N:(y + 1) * N],
                            start=(y == 0), stop=(y == YH - 1))
                    part = wk.tile([MUL, N], F32, name=f"part{l}", tag="part")
                    nc.vector.tensor_copy(part[:], pf[:])
                    part3 = part

            # ---- layer-3 combine: 8-way AllGather, sum pair halves ----
            ag3i = dp.tile([MUL, N], F32, name="ag3i")
            nc.sync.dma_start(ag3i[:], part3[:])
            ag3o = dp.tile([NCORES * MUL, N], F32, name="ag3o")
            nc.gpsimd.collective_compute(
                "AllGather", ALU.bypass,
                replica_groups=[list(range(NCORES))],
                ins=[ag3i.opt()], outs=[ag3o.opt()])
            agv = ag3o.opt().rearrange("(b h i) x -> h i b x", h=2, i=MUL)
            t3a = hd.tile([MUL, B * N], F32, name="t3a", tag="t3")
            t3b = hd.tile([MUL, B * N], F32, name="t3b", tag="t3")
            nc.sync.dma_start(
                t3a[:].rearrange("i (b x) -> i b x", b=B), agv[0])
            nc.sync.dma_start(
                t3b[:].rearrange("i (b x) -> i b x", b=B), agv[1])
            nc.vector.tensor_tensor(t3a[:], t3a[:], t3b[:], op=ALU.add)
            fT = gate_chain(t3a[:], MUL, B * N, mhead[:], "hd")

            # ---- head: 2x (linear + BN + lrelu), masked mean pool ----
            BN_ = B * N

            def bn_block(rhs_tiles, wts, bias, gbrow, cnt, nchunk, wchunkpool):
                """rhs_tiles: list of bf16 [K, BN_] inputs (chunked lhsT in
                wts); returns list of bf16 [128, BN_] outputs."""
                asb, a16 = [], []
                for m in range(nchunk):
                    aps = pA.tile([128, BN_], F32, name=f"aps{m}", tag="big")
                    for kk, rt in enumerate(rhs_tiles):
                        nc.tensor.matmul(
                            aps[:], wts[m][kk], rt[:],
                            start=(kk == 0), stop=(kk == len(rhs_tiles) - 1))
                    av = hd.tile([128, BN_], F32, name=f"av{m}", tag="av", bufs=3)
                    nc.vector.tensor_scalar(
                        av[:], aps[:], bias[:, m:m + 1], None, op0=ALU.add)
                    sq = hd.tile([128, BN_], BF16, name=f"sq{m}", tag="sq", bufs=3)
                    nc.scalar.activation(
                        sq[:], aps[:], AF.Square, bias=bias[:, m:m + 1])
                    a6 = hd.tile([128, BN_], BF16, name=f"a6{m}", tag="a6", bufs=3)
                    nc.vector.tensor_copy(a6[:], av[:])
                    asb.append(av)
                    a16.append((a6, sq))
                sps = pG.tile([1, BN_], F32, name="sps", tag="g")
                qps = pG.tile([1, BN_], F32, name="qps", tag="g")
                for m in range(nchunk):
                    nc.tensor.matmul(sps[:], ones128[:], a16[m][0][:],
                                     start=(m == 0), stop=(m == nchunk - 1))
                for m in range(nchunk):
                    nc.tensor.matmul(qps[:], ones128[:], a16[m][1][:],
                                     start=(m == 0), stop=(m == nchunk - 1))
                srow = hd.tile([1, BN_], F32, name="srow", tag="row", bufs=6)
                qrow = hd.tile([1, BN_], F32, name="qrow", tag="row", bufs=6)
                nc.vector.tensor_copy(srow[:], sps[:])
                nc.vector.tensor_copy(qrow[:], qps[:])
                mu = hd.tile([1, N], F32, name="mu", tag="r128", bufs=10)
                var = hd.tile([1, N], F32, name="var", tag="r128", bufs=10)
                nc.vector.tensor_tensor(
                    mu[:], srow[:, 0:N], srow[:, N:2 * N], op=ALU.add)
                nc.vector.tensor_tensor(
                    mu[:], mu[:], srow[:, 2 * N:3 * N], op=ALU.add)
                nc.vector.tensor_tensor(
                    mu[:], mu[:], srow[:, 3 * N:4 * N], op=ALU.add)
                nc.vector.tensor_tensor(
                    var[:], qrow[:, 0:N], qrow[:, N:2 * N], op=ALU.add)
                nc.vector.tensor_tensor(
                    var[:], var[:], qrow[:, 2 * N:3 * N], op=ALU.add)
                nc.vector.tensor_tensor(
                    var[:], var[:], qrow[:, 3 * N:4 * N], op=ALU.add)
                nc.vector.tensor_scalar_mul(mu[:], mu[:], 1.0 / cnt)
                nc.vector.tensor_scalar_mul(var[:], var[:], 1.0 / cnt)
                musq = hd.tile([1, N], F32, name="musq", tag="r128", bufs=10)
                nc.vector.tensor_tensor(musq[:], mu[:], mu[:], op=ALU.mult)
                nc.vector.tensor_tensor(
                    var[:], var[:], musq[:], op=ALU.subtract)
                inv = hd.tile([1, N], F32, name="inv", tag="r128", bufs=10)
                nc.scalar.activation(inv[:], var[:], AF.Ln, bias=epsv[:, 0:1])
                nc.scalar.activation(inv[:], inv[:], AF.Exp, scale=-0.5)
                svec = hd.tile([1, N], F32, name="svec", tag="r128", bufs=10)
                nc.vector.tensor_tensor(
                    svec[:], gbrow[:, 0:N], inv[:], op=ALU.mult)
                tvec = hd.tile([1, N], F32, name="tvec", tag="r128", bufs=10)
                nc.vector.tensor_tensor(tvec[:], mu[:], svec[:], op=ALU.mult)
                nc.vector.scalar_tensor_tensor(
                    tvec[:], tvec[:], -1.0, gbrow[:, N:2 * N],
                    op0=ALU.mult, op1=ALU.add)
                sw = hd.tile([1, BN_], F32, name="sw", tag="row", bufs=6)
                tw = hd.tile([1, BN_], F32, name="tw", tag="row", bufs=6)
                for b in range(B):
                    nc.vector.tensor_copy(sw[:, b * N:(b + 1) * N], svec[:])
                    nc.vector.tensor_copy(tw[:, b * N:(b + 1) * N], tvec[:])
                sB = pG.tile([128, BN_], F32, name="sB", tag="g")
                tB = pG.tile([128, BN_], F32, name="tB", tag="g")
                nc.tensor.matmul(sB[:], ones1[:], sw[:], start=True, stop=True)
                nc.tensor.matmul(tB[:], ones1[:], tw[:], start=True, stop=True)
                outs = []
                for m in range(nchunk):
                    nc.vector.tensor_tensor(
                        asb[m][:], asb[m][:], sB[:], op=ALU.mult)
                    nc.vector.tensor_tensor(
                        asb[m][:], asb[m][:], tB[:], op=ALU.add)
                    ho = hd.tile([128, BN_], BF16, name=f"ho{m}", tag="ho", bufs=4)
                    nc.scalar.activation(ho[:], asb[m][:], AF.Prelu, alpha=0.2)
                    outs.append(ho)
                return outs

            w1chunks = [[w1c[:, m * 128:(m + 1) * 128]] for m in range(2)]
            h1 = bn_block([fT], w1chunks, b1c, gb1, float(B * MID), 2, None)
            w2chunks = [[w2c[:, k * 128:(k + 1) * 128] for k in range(2)]]
            h2 = bn_block(h1, w2chunks, b2c, gb2, float(B * 128), 1, None)

            # masked mean pool -> out[b, o]
            h2m = hd.tile([128, BN_], BF16, name="h2m", tag="ho", bufs=4)
            nc.vector.tensor_tensor(h2m[:], h2[0][:], maskB[:], op=ALU.mult)
            pool = hd.tile([128, B], F32, name="pool", tag="pool")
            from concourse import mybir as _mb
            nc.vector.reduce_sum(
                pool[:], h2m[:].rearrange("p (b x) -> p b x", b=B),
                axis=_mb.AxisListType.X)
            nc.vector.tensor_tensor(pool[:], pool[:], mvB[:], op=ALU.mult)
            nc.sync.dma_start(out_d[:].rearrange("b o -> o b"), pool[:])

    nc.compile()
    return nc


def _ssp_chain(r, rw1, rw2, rw3_unused):
    grid = np.linspace(0.0, MAXR, NB)
    step = grid[1] - grid[0]
    x = (r[..., None] - grid) / step
    basis = np.where(np.abs(x) < 1.0, np.cos(0.5 * math.pi * x) ** 2, 0.0)

    def ssp(v):
        return (np.logaddexp(0, BETA * v) - math.log(2.0)) / BETA

    h = ssp(basis @ rw1 / math.sqrt(NB))
    h = ssp(h @ rw2 / math.sqrt(HID))
    return h


def _host_prep(inputs):
    import ml_dtypes
    BF = ml_dtypes.bfloat16
    f32 = np.float32

    f = {k: np.asarray(v) for k, v in inputs.items()}
    geometry = f["geometry"].astype(np.float64)
    features = f["features"].astype(np.int64)
    mask = f["mask"].astype(np.float64)
    emb = f["emb"].astype(np.float64)
    rw1, rw2, rw3 = (f[k].astype(np.float64) for k in ("rw1", "rw2", "rw3"))
    W1, b1 = f["W1"].astype(np.float64), f["b1"].astype(np.float64)
    W2, b2 = f["W2"].astype(np.float64), f["b2"].astype(np.float64)
    g1, be1 = f["g1"].astype(np.float64), f["be1"].astype(np.float64)
    g2, be2 = f["g2"].astype(np.float64), f["be2"].astype(np.float64)

    # grid / hat coefficients
    rg = np.linspace(0.0, RMAX, GRID_N)
    c = rg ** 2
    cl = np.empty(GRID_N); cr = np.empty(GRID_N)
    cl[1:] = c[:-1]; cl[0] = c[0] - 1.0
    cr[:-1] = c[1:]; cr[-1] = c[-1] + 1.0
    sl = 1.0 / (c - cl)
    sr = 1.0 / (cr - c)
    slq = np.asarray(sl, BF).astype(np.float64)     # quantized, used in PE
    coef = np.zeros((GRID_N, 3))
    coef[:, 0] = -cl * slq                           # A = psum + coef0
    coef[:, 1] = -sr / slq                           # B = psum*coef1+coef2
    coef[:, 2] = sr * cr

    ftab = np.zeros((GRID_N, NL * HID))
    for l in range(NL):
        ftab[:, l * HID:(l + 1) * HID] = _ssp_chain(rg, rw1[l], rw2[l], None)

    wgh = np.zeros((MUL, NL * MUL * HID))
    gfac = Y0 / math.sqrt(HID)
    for l in range(NL):
        r3 = rw3[l].reshape(HID, MUL, MUL)           # [h, i, j]
        wgl = gfac * r3.transpose(2, 1, 0)           # [j, i, h]
        wgl = wgl.reshape(MUL, MUL * HID)
        wgh[:, l * MUL * HID:(l + 1) * MUL * HID] = wgl

    norms = (geometry ** 2).sum(axis=-1)             # [B, N]
    f0_all = emb[features[..., 0]]                   # [B, N, EMB]

    w2c = np.zeros((128, MID))
    for k in range(2):
        w2c[:, k * 128:(k + 1) * 128] = W2[k * 128:(k + 1) * 128, :]

    msum = mask.sum(axis=1)                          # [B]
    shared = {
        "slrow": np.asarray(sl, BF).reshape(1, GRID_N),
        "coef": coef.astype(f32),
        "ftab": np.asarray(ftab, BF),
        "wg": np.asarray(wgh, BF),
        "mhead": np.repeat(mask.reshape(1, B * N), MUL, axis=0).astype(f32),
        "w1c": np.asarray(W1, BF),
        "b1c": b1.reshape(2, 128).T.astype(f32),
        "w2c": np.asarray(w2c, BF),
        "b2c": b2.reshape(128, 1).astype(f32),
        "gb1": np.concatenate([g1, be1]).reshape(1, 2 * N).astype(f32),
        "gb2": np.concatenate([g2, be2]).reshape(1, 2 * N).astype(f32),
        "ones128": np.ones((128, 1), BF),
        "ones1": np.ones((1, 128), f32),
        "maskB": np.repeat(mask.reshape(1, B * N), 128, axis=0).astype(BF),
        "mvB": np.repeat((1.0 / msum).reshape(1, B), 128, axis=0).astype(f32),
        "epsv": np.full((1, 1), 1e-5, f32),
    }

    in_maps = []
    for core in range(NCORES):
        b = core // 2
        y0 = (core % 2) * YH
        geoYm = np.zeros((5, YH))
        geoYm[0:3] = -2.0 * geometry[b, y0:y0 + YH].T
        geoYm[3] = norms[b, y0:y0 + YH]
        geoYm[4] = 1.0
        geoXm = np.zeros((5, N))
        geoXm[0:3] = geometry[b].T
        geoXm[3] = 1.0
        geoXm[4] = norms[b]
        fm0 = (f0_all[b] * mask[b][:, None] * SQN).T    # [32, N]
        m = dict(shared)
        m["geoY"] = geoYm.astype(f32)
        m["geoX"] = geoXm.astype(f32)
        m["fm0"] = np.asarray(fm0[:, y0:y0 + YH], BF)
        m["msqn"] = np.repeat(
            (mask[b, y0:y0 + YH] * SQN).reshape(YH, 1), MUL, axis=1
        ).astype(f32)
        m["ident64"] = np.eye(YH, dtype=f32)
        in_maps.append(m)
    return in_maps


def run(inputs, trace=False):
    global _cached
    from concourse import bass_utils
    if _cached is None:
        _cached = _build()
    nc = _cached
    in_maps = _host_prep(inputs)
    res = bass_utils.run_bass_kernel_spmd(
        nc, in_maps, core_ids=list(range(NCORES)), trace=trace)
    return res


def kernel(**inputs):
    res = run(inputs, trace=False)
    return np.asarray(res.results[0]["out"], dtype=np.float32)
